# revision 12
# baseline (speedup 1.0000x reference)
"""Trainium2 Bass kernel for nn_BandpassFilter (cascaded 1st-order Butterworth
highpass+lowpass IIR over time, batch 128 x T 262144, f32).

Math: the reference cascade
    y1[t] = bh0*x[t] + bh1*x[t-1] - ah1*y1[t-1]   (highpass: bh1 = -bh0)
    y2[t] = bl0*y1[t] + bl1*y1[t-1] - al1*y2[t-1] (lowpass:  bl1 = +bl0)
is the LTI transfer  H(z) = gain*bh0*bl0 * (1 - z^-2) / ((1+ah1 z^-1)(1+al1 z^-1)).
We evaluate it as
    d[t] = x[t] - x[t-2]
    v[t] = rho_h*v[t-1] + d[t]        (rho_h = -ah1)
    w[t] = rho_l*w[t-1] + v[t]        (rho_l = -al1)
    y[t] = C*w[t],  C = gain*bh0*bl0
The two pole recurrences map onto the hardware tensor_tensor_scan instruction
(state = data0*state + data1 along the free axis, one recurrence per partition).

Distribution: data-parallel over 8 cores (16 batch rows each). Inside a core,
each row is split into SEG=8 time segments so all 128 SBUF partitions are busy;
since [16, 262144] row-major == [128, 32768] row-major, the per-core x/y DRAM
tensors are declared [128, 32768] and partition p holds segment (p % 8) of row
(p // 8). Segments are made independent by a warm-up halo: the poles
|rho| <= 0.91, so rho^HALO (HALO=512) ~ 1e-22 — scanning HALO real samples from
a zero state reproduces the exact running state to below f32 round-off.
Chunks within a segment chain exactly via the scan's `initial` operand.
"""

import sys

import numpy as np

if "/opt/trn_rl_repo" not in sys.path:
    sys.path.insert(0, "/opt/trn_rl_repo")

from contextlib import ExitStack


def _coeffs(center_freq, bandwidth, gain, sample_rate):
    """First-order Butterworth coefficients, mirroring reference.py in f32."""
    f32 = np.float32
    nyq = float(sample_rate) / 2.0
    low_wn = f32((f32(center_freq) - f32(bandwidth) / f32(2.0)) / nyq)
    high_wn = f32((f32(center_freq) + f32(bandwidth) / f32(2.0)) / nyq)

    Kh = np.tan(f32(np.pi * low_wn / 2.0), dtype=f32)
    ah1 = f32((Kh - f32(1.0)) / (Kh + f32(1.0)))
    bh0 = f32(f32(1.0) / (Kh + f32(1.0)))

    Kl = np.tan(f32(np.pi * high_wn / 2.0), dtype=f32)
    al1 = f32((Kl - f32(1.0)) / (Kl + f32(1.0)))
    bl0 = f32(Kl / (Kl + f32(1.0)))

    rho_h = f32(-ah1)
    rho_l = f32(-al1)
    C = f32(f32(gain) * bh0 * bl0)
    return float(rho_h), float(rho_l), float(C)


def build_nc(rho_h, rho_l, C, P=128, S=32768, SEG=8, F=4096, HALO=512):
    """Per-core Bass program. x,y: [P, S] in DRAM; partition p = (row, seg)."""
    import concourse.bacc as bacc
    import concourse.mybir as mybir
    import concourse.tile as tile

    NCH = S // F
    assert F * NCH == S and P <= 128 and P % SEG == 0

    nc = bacc.Bacc("TRN2", target_bir_lowering=False)
    dt = mybir.dt.float32
    mult = mybir.AluOpType.mult
    add = mybir.AluOpType.add

    x_in = nc.dram_tensor("x", [P, S], dt, kind="ExternalInput")
    y_out = nc.dram_tensor("y", [P, S], dt, kind="ExternalOutput")
    x2 = x_in.ap()
    y2 = y_out.ap()

    with ExitStack() as ctx:
        tc = ctx.enter_context(tile.TileContext(nc))
        const_pool = ctx.enter_context(tc.tile_pool(name="const", bufs=1))
        halo_pool = ctx.enter_context(tc.tile_pool(name="halo", bufs=1))
        x_pool = ctx.enter_context(tc.tile_pool(name="xp", bufs=3))
        do_pool = ctx.enter_context(tc.tile_pool(name="dout", bufs=2))
        v_pool = ctx.enter_context(tc.tile_pool(name="vp", bufs=2))
        w_pool = ctx.enter_context(tc.tile_pool(name="wp", bufs=2))

        rho_h_t = const_pool.tile([P, F], dt, tag="rho_h")
        rho_l_t = const_pool.tile([P, F], dt, tag="rho_l")
        nc.vector.memset(rho_h_t[:], rho_h)
        nc.vector.memset(rho_l_t[:], rho_l)

        # Per-partition mask: 0.0 where p % SEG == 0 (true sequence start,
        # zero initial state), 1.0 elsewhere. Built via a free-strided memset
        # in one partition, then DMA-scattered across partitions.
        mrow = const_pool.tile([1, P], dt, tag="mrow")
        nc.vector.memset(mrow[:], 1.0)
        mrow_v = mrow[:].rearrange("p (a b) -> p a b", b=SEG)
        nc.vector.memset(mrow_v[:, :, 0:1], 0.0)
        mask = const_pool.tile([P, 1], dt, tag="mask")
        nc.sync.dma_start(mask[:, 0:1], mrow[0:1, 0:P])

        # Segment warm-up: scan the HALO samples preceding each segment from a
        # zero state. Partition p's predecessor data is partition p-1's tail
        # (junk-but-finite for p % SEG == 0; those warm-up results are zeroed
        # by `mask` below, matching the reference's zero initial conditions).
        xh = halo_pool.tile([P, HALO + 2], dt, tag="xh")
        nc.vector.memset(xh[0:1, :], 0.0)
        nc.sync.dma_start(xh[1:P, :], x2[0 : P - 1, S - (HALO + 2) : S])
        sub = mybir.AluOpType.subtract
        dh = halo_pool.tile([P, HALO], dt, tag="dh")
        nc.vector.scalar_tensor_tensor(
            dh[:], xh[:, 2 : HALO + 2], 1.0, xh[:, 0:HALO], op0=mult, op1=sub
        )
        vh = halo_pool.tile([P, HALO], dt, tag="vh")
        nc.vector.tensor_tensor_scan(vh[:], rho_h_t[:, 0:HALO], dh[:], 0.0, mult, add)
        wh = halo_pool.tile([P, HALO], dt, tag="wh")
        nc.vector.tensor_tensor_scan(wh[:], rho_l_t[:, 0:HALO], vh[:], 0.0, mult, add)
        vh_i = halo_pool.tile([P, 1], dt, tag="vh_i")
        nc.vector.tensor_scalar_mul(vh_i[:], vh[:, HALO - 1 : HALO], mask[:, 0:1])
        wh_i = halo_pool.tile([P, 1], dt, tag="wh_i")
        nc.vector.tensor_scalar_mul(wh_i[:], wh[:, HALO - 1 : HALO], mask[:, 0:1])

        v_prev, w_prev, pcol = vh_i, wh_i, 1
        for c in range(NCH):
            xc = x_pool.tile([P, F + 2], dt, tag="xc")
            if c == 0:
                nc.vector.tensor_scalar_mul(
                    xc[:, 0:2], xh[:, HALO : HALO + 2], mask[:, 0:1]
                )
                nc.sync.dma_start(xc[:, 2 : F + 2], x2[:, 0:F])
            else:
                nc.sync.dma_start(xc[:], x2[:, c * F - 2 : c * F + F])

            dc = do_pool.tile([P, F], dt, tag="dout", name=f"d{c}")
            nc.vector.scalar_tensor_tensor(
                dc[:], xc[:, 2 : F + 2], 1.0, xc[:, 0:F], op0=mult, op1=sub
            )

            vc = v_pool.tile([P, F], dt, tag="vc", name=f"v{c}")
            nc.vector.tensor_tensor_scan(
                vc[:], rho_h_t[:], dc[:], v_prev[:, pcol - 1 : pcol], mult, add
            )
            wc = w_pool.tile([P, F], dt, tag="wc", name=f"w{c}")
            nc.vector.tensor_tensor_scan(
                wc[:], rho_l_t[:], vc[:], w_prev[:, pcol - 1 : pcol], mult, add
            )

            oc = do_pool.tile([P, F], dt, tag="dout", name=f"o{c}")
            nc.scalar.mul(oc[:], wc[:], C)
            nc.sync.dma_start(y2[:, c * F : (c + 1) * F], oc[:])

            v_prev, w_prev, pcol = vc, wc, F

    nc.compile()
    return nc


TRACE = False
LAST_EXEC_TIME_NS = None
LAST_RESULT = None


def kernel(x, center_freq, bandwidth, gain, sample_rate):
    global LAST_EXEC_TIME_NS, LAST_RESULT
    from concourse.bass_utils import run_bass_kernel_spmd

    x = np.ascontiguousarray(np.asarray(x, dtype=np.float32))
    B, T = x.shape  # 128, 262144
    n_cores = 8
    rows = B // n_cores  # 16
    SEG = 8
    P = rows * SEG  # 128
    S = T // SEG  # 32768

    rho_h, rho_l, C = _coeffs(
        float(np.asarray(center_freq)),
        float(np.asarray(bandwidth)),
        float(np.asarray(gain)),
        float(np.asarray(sample_rate)),
    )

    nc = build_nc(rho_h, rho_l, C, P=P, S=S, SEG=SEG, F=4096, HALO=512)

    in_maps = [
        {"x": x[i * rows : (i + 1) * rows].reshape(P, S)} for i in range(n_cores)
    ]
    res = run_bass_kernel_spmd(
        nc, in_maps, core_ids=list(range(n_cores)), trace=TRACE
    )
    LAST_EXEC_TIME_NS = res.exec_time_ns
    LAST_RESULT = res
    out = np.concatenate(
        [res.results[i]["y"].reshape(rows, T) for i in range(n_cores)], axis=0
    )
    return out


if __name__ == "__main__":
    rng = np.random.default_rng(0)
    x = rng.standard_normal((128, 262144), dtype=np.float32)
    y = kernel(x, np.float32(1000.0), np.float32(500.0), np.float32(1.0), 48000)
    print(y.shape, y.dtype, float(np.abs(y).mean()))
